# revision 10
# baseline (speedup 1.0000x reference)
"""AttMaxPool2D (2x2 softmax-attention pooling) Trainium2 Bass kernel.

out[b, wo, ho, c] = sum_i p_i * exp(t*p_i) / sum_i exp(t*p_i)
over the 4 elements p_i of each 2x2 window of x[b, :, :, c] (softmax-
weighted pooling; jax.nn.softmax's max-subtraction cancels analytically).

Sharding: pure batch data-parallel, 8 cores x 4 images, no collectives.

Shipped configuration (SHIP_CONFIG: variant="f16", f16io=True): f16
end-to-end --
 - x ships/stores in HBM as f16 (16.8MB/core), output as f16 (4.2MB);
   fp32 only in PSUM accumulation and the recip/final-mul drain.
 - SBUF tiles hold [w:128(partitions), (h:64, c:128)(free)] double
   slabs (dslab=True): one ACT exp + one DVE mul instruction per 64
   h-rows (fewer per-op inits on the bottleneck DVE; ~0.7% in A/B).
 - e = exp(t*x) on ScalarE (f16 -> f16, 1x rate: ~57us/core busy).
 - pe = x*e on VectorE as f16 tensor_tensor -> runs in the DVE 2x_1P
   packed mode (the key win over the old bf16-in/f32r config whose
   mixed-dtype mul ran 1x).
 - 2x2-window contraction on the PE as f16 matmuls against a 0/1
   pair-sum weight matrix (full rate + FWL), PSUM-accumulating h-pairs.
 - r = 1/den (custom-DVE fast recip, fp32 from PSUM), out = num*r on
   VectorE (f16 out).
 - No GPSIMD anywhere (shares/locks the DVE SBUF port; measured harmful
   in every layout).

Engine model per core per rep (DVE @0.96GHz is the bottleneck and is
provably at its floor for this decomposition):
  DVE: pe-mul 33.7k cyc (2 elem/cyc packed; port floor) + recip 17.3k
       + final-mul 17.3k (both 1 elem/cyc through the single 32-bit
       DVE<->PSUM read port; floor) = 68.4k cyc = 71.2us
  ACT: exp 16x(4096+352)/1.2GHz = 59.3us   PE: ~55us   DMA: 21MB ~59us
Measured (paired pipelined-slope reps-delta, see test.py):
  shipped f16 e2e:           71.9us/rep, rel err 8.9e-4
  prior   f32r+gp0+bf16in:   ~96-104us/rep, rel err 3.7e-3
  f32r+gp0, fp32 io:         ~125-127us/rep (= fp32-byte DMA floor)
  no-PE same-partition pair layout: ~210us/rep (all-DVE; rejected)
Rejected beyond-floor ideas (all dead-ends, see session notes): ACT
recip offload (exp/recip table sets can't coexist, 2.7us switch), DMA
PSUM drain (no route), GPSIMD anything (port lock), 2x_2P custom recip
(needs 16 slices), finite-difference num (2x ACT or f32-cancellation),
pairwise sigmoid/swish trees (not the same function / more ACT work).
A single reps=1 execution additionally pays ~600us dispatch+launch
overhead through the axon/PJRT stack; that term is environment-, not
kernel-, determined.
"""

import numpy as np
from contextlib import ExitStack

N_CORES = 8
B, W, H, C = 32, 128, 128, 128
BS = B // N_CORES            # images per core
HCHUNK = 16                  # h rows per slab
NPP = H // (2 * HCHUNK)      # psum iterations per image (h-chunk pairs)
WO, HO = W // 2, H // 2
FREE = HCHUNK * C            # slab free size (2048 f32)
PFREE = (HCHUNK // 2) * C    # psum free size (1024 f32)

# rows of each slab's pe-multiply done on GpSimd (of HCHUNK)
GP_PE_ROWS_F32 = 0
GP_PE_ROWS_F32R = 8


def _build(temp: float, reps: int = 1, variant: str = "fp32",
           dma_only: bool = False, no_pe: bool = False,
           gp_rows_ovr: int = None, direct_den: bool = False,
           hchunk: int = HCHUNK, sp_gp: bool = False,
           xbufs: int = 5, ebufs: int = 4, pebufs: int = 4, sbufs: int = 4,
           psbufs: int = None, out_alt: bool = False, robufs: int = 3,
           fused: bool = False, narrow_w: bool = False,
           ncores: int = N_CORES, f16io: bool = False,
           bf16io: bool = False, bf16in: bool = False,
           qbal: bool = False, split_ps: bool = False,
           in3q: bool = False, dslab: bool = False):
    import concourse.bacc as bacc
    import concourse.tile as tile
    from concourse import mybir

    f32 = mybir.dt.float32
    f16 = mybir.dt.float16
    f32r = mybir.dt.float32r
    use_f32r = variant in ("f32r", "f16")
    edt = {"f32r": f32r, "f16": f16}.get(variant, f32)
    bf16 = mybir.dt.bfloat16
    xdt = bf16 if (bf16io or bf16in) else (f16 if f16io else f32)
    odt = bf16 if bf16io else (f16 if f16io else f32)
    bs = B // ncores

    free = hchunk * C
    pfree = (hchunk // 2) * C
    npp = H // (2 * hchunk)
    nq = pfree // 512
    if psbufs is None:
        psbufs = max(1, 8 // (2 * (pfree // 512 * 1)))
        psbufs = min(psbufs, 2)

    nc = bacc.Bacc("TRN2", target_bir_lowering=False, debug=False,
                   num_devices=ncores)
    x_ap = nc.dram_tensor("x", [bs, W, H, C], xdt, kind="ExternalInput").ap()
    w_ap = nc.dram_tensor("wmat", [2, W, 128], edt,
                          kind="ExternalInput").ap()
    out_ap = nc.dram_tensor("out", [bs, WO, HO, C], odt,
                            kind="ExternalOutput").ap()

    with tile.TileContext(nc) as tc:
        with ExitStack() as ctx:
            wpool = ctx.enter_context(tc.tile_pool(name="w", bufs=1))
            xpool = ctx.enter_context(tc.tile_pool(name="x", bufs=xbufs))
            epool = ctx.enter_context(tc.tile_pool(name="e", bufs=ebufs))
            pepool = ctx.enter_context(tc.tile_pool(name="pe", bufs=pebufs))
            spool = ctx.enter_context(tc.tile_pool(name="s", bufs=sbufs))
            rpool = ctx.enter_context(tc.tile_pool(name="r", bufs=robufs))
            opool = ctx.enter_context(tc.tile_pool(name="o", bufs=robufs))
            pspool = ctx.enter_context(
                tc.tile_pool(name="ps", bufs=psbufs, space="PSUM"))

            wm = wpool.tile([W, 256], edt)
            nc.sync.dma_start(wm[:, 0:128], w_ap[0])
            nc.sync.dma_start(wm[:, 128:256], w_ap[1])

            gp_rows = GP_PE_ROWS_F32R if use_f32r else GP_PE_ROWS_F32
            if gp_rows_ovr is not None:
                gp_rows = gp_rows_ovr
            if fused:
                # hchunk=32 slabs, single big ops, [128,1024] psum granularity
                assert hchunk == 32 and variant == "fp32"
                for _rep in range(reps):
                    for b in range(bs):
                        for sl in range(4):
                            t3 = xpool.tile([128, free], xdt, tag="t",
                                            name="t3").rearrange(
                                "p (h c) -> p h c", h=hchunk)
                            eng = nc.sync if (sl % 2 == 0) else nc.scalar
                            eng.dma_start(
                                t3,
                                x_ap[b, :, sl * hchunk:(sl + 1) * hchunk, :])
                            e3 = epool.tile([128, free], f32, tag="e",
                                            name="e3").rearrange(
                                "p (h c) -> p h c", h=hchunk)
                            nc.scalar.activation(
                                e3, t3, mybir.ActivationFunctionType.Exp,
                                scale=float(temp))
                            pe3 = pepool.tile([128, free], f32, tag="pe",
                                              name="pe3").rearrange(
                                "p (h c) -> p h c", h=hchunk)
                            nc.vector.tensor_mul(pe3, t3, e3)
                            sE = spool.tile([128, free // 2], f32, tag="sE",
                                            name="sE").rearrange(
                                "p (h c) -> p h c", h=hchunk // 2)
                            sP = spool.tile([128, free // 2], f32, tag="sP",
                                            name="sP").rearrange(
                                "p (h c) -> p h c", h=hchunk // 2)
                            nc.vector.tensor_add(
                                sE, e3[:, 0::2, :], e3[:, 1::2, :])
                            if sp_gp:
                                nc.gpsimd.tensor_add(
                                    sP, pe3[:, 0::2, :], pe3[:, 1::2, :])
                            else:
                                nc.vector.tensor_add(
                                    sP, pe3[:, 0::2, :], pe3[:, 1::2, :])
                            den_ps = pspool.tile([128, 1024], f32)
                            num_ps = pspool.tile([128, 1024], f32)
                            for g in range(2):
                                wm_g = wm[:, g * 128:(g + 1) * 128]
                                for q in range(2):
                                    h0 = g * 8 + q * 4
                                    if narrow_w:
                                        # [128,64] weight + dst partition
                                        # offset: halves each LDW (fp32 has
                                        # no fast-weight-load)
                                        ps_sl = (slice(g * 64, (g + 1) * 64),
                                                 slice(q * 512, (q + 1) * 512))
                                        nc.tensor.matmul(
                                            den_ps[ps_sl], wm[:, 0:64],
                                            sE[:, h0:h0 + 4, :],
                                            start=True, stop=True)
                                        nc.tensor.matmul(
                                            num_ps[ps_sl], wm[:, 0:64],
                                            sP[:, h0:h0 + 4, :],
                                            start=True, stop=True)
                                        continue
                                    ps_sl = (slice(0, 128),
                                             slice(q * 512, (q + 1) * 512))
                                    nc.tensor.matmul(
                                        den_ps[ps_sl], wm_g,
                                        sE[:, h0:h0 + 4, :],
                                        start=(g == 0), stop=(g == 1))
                                    nc.tensor.matmul(
                                        num_ps[ps_sl], wm_g,
                                        sP[:, h0:h0 + 4, :],
                                        start=(g == 0), stop=(g == 1))
                            r = rpool.tile([128, 1024], f32)
                            nc.vector.reciprocal_approx_fast(r[:], den_ps[:])
                            o = opool.tile([128, 1024], odt)
                            nc.vector.tensor_mul(o[:], num_ps[:], r[:])
                            o3 = o.rearrange("p (h c) -> p h c", h=8)
                            for g in range(2):
                                ho0 = sl * 16 + g * 8
                                nc.sync.dma_start(
                                    out_ap[b, :, ho0:ho0 + 8, :],
                                    o3[g * 64:(g + 1) * 64, :, :])
            for _rep in range(reps if not fused else 0):
                for b in range(bs):
                    for pp in range(npp):
                        if split_ps:
                            den_h = [pspool.tile([128, 1024], f32,
                                                 name=f"dh{h}")
                                     for h in range(2)]
                            num_h = [pspool.tile([128, 1024], f32,
                                                 name=f"nh{h}")
                                     for h in range(2)]
                            den_ps = num_ps = None
                        else:
                            den_ps = pspool.tile([128, pfree], f32)
                            num_ps = pspool.tile([128, pfree], f32)
                        if dslab:
                            # one [128, 2*free] slab per pp: single ACT
                            # exp + single DVE mul over both j2 halves
                            # (fewer per-op inits on the bottleneck DVE)
                            assert use_f32r and not dma_only
                            assert gp_rows == 0 and not split_ps
                            t3d = xpool.tile([128, 2 * free], xdt, tag="t",
                                             name="t3").rearrange(
                                "p (h c) -> p h c", h=2 * hchunk)
                            for j2 in range(2):
                                hp = 2 * pp + j2
                                eng = (nc.sync if (hp % 2 == 0)
                                       else nc.scalar)
                                eng.dma_start(
                                    t3d[:, j2 * hchunk:(j2 + 1) * hchunk,
                                        :],
                                    x_ap[b, :,
                                         hp * hchunk:(hp + 1) * hchunk, :])
                            e3d = epool.tile([128, 2 * free], edt, tag="e",
                                             name="e3").rearrange(
                                "p (h c) -> p h c", h=2 * hchunk)
                            nc.scalar.activation(
                                e3d, t3d, mybir.ActivationFunctionType.Exp,
                                scale=float(temp))
                            pe3d = pepool.tile([128, 2 * free], edt,
                                               tag="pe",
                                               name="pe3").rearrange(
                                "p (h c) -> p h c", h=2 * hchunk)
                            nc.vector.tensor_mul(pe3d, t3d, e3d)
                        for j2 in range(2):
                            hp = 2 * pp + j2
                            hoff = 0
                            if dslab:
                                e3, pe3 = e3d, pe3d
                                hoff = j2 * hchunk
                            else:
                                t3 = xpool.tile([128, free], xdt, tag="t",
                                                name="t3").rearrange(
                                    "p (h c) -> p h c", h=hchunk)
                                if in3q:
                                    eng = [nc.sync, nc.scalar,
                                           nc.gpsimd][(b * npp * 2 + hp)
                                                      % 3]
                                else:
                                    eng = (nc.sync if (hp % 2 == 0)
                                           else nc.scalar)
                                eng.dma_start(
                                    t3,
                                    x_ap[b, :,
                                         hp * hchunk:(hp + 1) * hchunk, :])
                                if dma_only:
                                    continue
                                e3 = epool.tile([128, free], edt, tag="e",
                                                name="e3").rearrange(
                                    "p (h c) -> p h c", h=hchunk)
                                nc.scalar.activation(
                                    e3, t3,
                                    mybir.ActivationFunctionType.Exp,
                                    scale=float(temp))
                                pe3 = pepool.tile([128, free], edt,
                                                  tag="pe",
                                                  name="pe3").rearrange(
                                    "p (h c) -> p h c", h=hchunk)
                                k = hchunk - gp_rows
                                nc.vector.tensor_mul(
                                    pe3[:, :k, :], t3[:, :k, :],
                                    e3[:, :k, :])
                                if gp_rows:
                                    nc.gpsimd.tensor_mul(
                                        pe3[:, k:, :], t3[:, k:, :],
                                        e3[:, k:, :])
                            wm_j = wm[:, j2 * 128:(j2 + 1) * 128]
                            if use_f32r:
                                for q in range(nq):
                                    for dh in range(2):
                                        h0 = hoff + q * 8 + dh
                                        h1 = hoff + q * 8 + 8
                                        if split_ps:
                                            dtile = den_h[q // 2]
                                            ntile = num_h[q // 2]
                                            ps_sl = (slice(0, 128),
                                                     slice((q % 2) * 512,
                                                           (q % 2 + 1) * 512))
                                        else:
                                            dtile = den_ps
                                            ntile = num_ps
                                            ps_sl = (slice(0, 128),
                                                     slice(q * 512,
                                                           (q + 1) * 512))
                                        st = (j2 == 0 and dh == 0)
                                        sp = (j2 == 1 and dh == 1)
                                        nc.tensor.matmul(
                                            dtile[ps_sl], wm_j,
                                            e3[:, h0:h1:2, :],
                                            start=st, stop=sp)
                                        nc.tensor.matmul(
                                            ntile[ps_sl], wm_j,
                                            pe3[:, h0:h1:2, :],
                                            start=st, stop=sp)
                            else:
                                sE = spool.tile([128, pfree], f32, tag="sE",
                                                name="sE").rearrange(
                                    "p (h c) -> p h c", h=hchunk // 2)
                                sP = spool.tile([128, pfree], f32, tag="sP",
                                                name="sP").rearrange(
                                    "p (h c) -> p h c", h=hchunk // 2)
                                if not direct_den:
                                    nc.vector.tensor_add(
                                        sE, e3[:, 0::2, :], e3[:, 1::2, :])
                                if sp_gp:
                                    nc.gpsimd.tensor_add(
                                        sP, pe3[:, 0::2, :], pe3[:, 1::2, :])
                                else:
                                    nc.vector.tensor_add(
                                        sP, pe3[:, 0::2, :], pe3[:, 1::2, :])
                                if no_pe:
                                    ho0 = hp * (hchunk // 2)
                                    nc.sync.dma_start(
                                        out_ap[b, :,
                                               ho0:ho0 + hchunk // 2, :],
                                        sE[0:64, :, :])
                                    continue
                                for q in range(nq):
                                    ps_sl = (slice(0, 128),
                                             slice(q * 512, (q + 1) * 512))
                                    q0, q1 = q * 4, (q + 1) * 4
                                    if direct_den:
                                        for dh in range(2):
                                            h0 = q * 8 + dh
                                            h1 = q * 8 + 8
                                            nc.tensor.matmul(
                                                den_ps[ps_sl], wm_j,
                                                e3[:, h0:h1:2, :],
                                                start=(j2 == 0 and dh == 0),
                                                stop=(j2 == 1 and dh == 1))
                                    else:
                                        nc.tensor.matmul(
                                            den_ps[ps_sl], wm_j,
                                            sE[:, q0:q1, :],
                                            start=(j2 == 0), stop=(j2 == 1))
                                    nc.tensor.matmul(
                                        num_ps[ps_sl], wm_j, sP[:, q0:q1, :],
                                        start=(j2 == 0), stop=(j2 == 1))
                        if no_pe:
                            continue
                        if dma_only:
                            for j2 in range(2):
                                ho0 = pp * hchunk + j2 * (hchunk // 2)
                                nc.sync.dma_start(
                                    out_ap[b, :, ho0:ho0 + hchunk // 2, :],
                                    t3[j2 * 64:(j2 + 1) * 64,
                                       0:hchunk // 2, :])
                            continue
                        if split_ps:
                            # per-half drain: recip/final of half A overlap
                            # half B's matmuls and release PSUM early
                            for hh in range(2):
                                r = rpool.tile([128, 1024], f32,
                                               name=f"r{hh}")
                                nc.vector.reciprocal_approx_fast(
                                    r[:], den_h[hh][:])
                                o = opool.tile([128, 1024], odt,
                                               name=f"o{hh}")
                                nc.vector.tensor_mul(o[:], num_h[hh][:],
                                                     r[:])
                                o3h = o.rearrange("p (h c) -> p h c", h=8)
                                for j2 in range(2):
                                    ho0 = (2 * pp + j2) * (hchunk // 2) \
                                        + hh * 8
                                    oeng = nc.gpsimd if qbal else nc.sync
                                    oeng.dma_start(
                                        out_ap[b, :, ho0:ho0 + 8, :],
                                        o3h[j2 * 64:(j2 + 1) * 64, :, :])
                            continue
                        r = rpool.tile([128, pfree], f32)
                        nc.vector.reciprocal_approx_fast(r[:], den_ps[:])
                        o = opool.tile([128, pfree], odt)
                        nc.vector.tensor_mul(o[:], num_ps[:], r[:])
                        o3 = o.rearrange("p (h c) -> p h c", h=hchunk // 2)
                        for j2 in range(2):
                            ho0 = pp * hchunk + j2 * (hchunk // 2)
                            if qbal:
                                oeng = nc.gpsimd
                            else:
                                oeng = (nc.scalar
                                        if (out_alt and j2 == 1) else nc.sync)
                            oeng.dma_start(
                                out_ap[b, :, ho0:ho0 + hchunk // 2, :],
                                o3[j2 * 64:(j2 + 1) * 64, :, :])
    nc.compile()
    return nc


def _build_diag(temp: float, reps: int = 1, mode: str = "actonly",
                ncores: int = N_CORES, xbufs: int = 4, ebufs: int = 3):
    """Diagnostic bodies matching the shipped kernel's DMA pattern.
    mode=actonly: in-DMA (bf16) + ACT exp only -> measures ACT
    throughput overlapped with input DMA.  mode=dmabf: in-DMA (bf16) +
    out-DMA (f32, from a memset tile) -> ship's byte traffic, zero
    compute."""
    import concourse.bacc as bacc
    import concourse.tile as tile
    from concourse import mybir

    f32 = mybir.dt.float32
    f32r = mybir.dt.float32r
    bf16 = mybir.dt.bfloat16
    bs = B // ncores
    hchunk = 32
    free = hchunk * C
    pfree = (hchunk // 2) * C
    npp = H // (2 * hchunk)

    nc = bacc.Bacc("TRN2", target_bir_lowering=False, debug=False,
                   num_devices=ncores)
    x_ap = nc.dram_tensor("x", [bs, W, H, C], bf16,
                          kind="ExternalInput").ap()
    out_ap = nc.dram_tensor("out", [bs, WO, HO, C], f32,
                            kind="ExternalOutput").ap()
    with tile.TileContext(nc) as tc:
        with ExitStack() as ctx:
            xpool = ctx.enter_context(tc.tile_pool(name="x", bufs=xbufs))
            epool = ctx.enter_context(tc.tile_pool(name="e", bufs=ebufs))
            opool = ctx.enter_context(tc.tile_pool(name="o", bufs=1))
            od = opool.tile([128, pfree], f32)
            nc.vector.memset(od, 1.0)
            for _rep in range(reps):
                for b in range(bs):
                    for pp in range(npp):
                        for j2 in range(2):
                            hp = 2 * pp + j2
                            t3 = xpool.tile([128, free], bf16, tag="t",
                                            name="t3").rearrange(
                                "p (h c) -> p h c", h=hchunk)
                            eng = nc.sync if (hp % 2 == 0) else nc.scalar
                            eng.dma_start(
                                t3,
                                x_ap[b, :, hp * hchunk:(hp + 1) * hchunk, :])
                            if mode == "actonly":
                                e3 = epool.tile([128, free], f32r, tag="e",
                                                name="e3").rearrange(
                                    "p (h c) -> p h c", h=hchunk)
                                nc.scalar.activation(
                                    e3, t3,
                                    mybir.ActivationFunctionType.Exp,
                                    scale=float(temp))
                        if mode == "dmabf":
                            o3 = od.rearrange("p (h c) -> p h c",
                                              h=hchunk // 2)
                            for j2 in range(2):
                                ho0 = pp * hchunk + j2 * (hchunk // 2)
                                nc.sync.dma_start(
                                    out_ap[b, :, ho0:ho0 + hchunk // 2, :],
                                    o3[j2 * 64:(j2 + 1) * 64, :, :])
                    if mode == "actonly":
                        # minimal out write to keep the NEFF valid
                        nc.sync.dma_start(out_ap[b, 0:64, 0:1, :],
                                          od[0:64, 0:128].rearrange(
                                              "p (h c) -> p h c", h=1))
    nc.compile()
    return nc


def _build_nope(temp: float, reps: int = 1, ncores: int = N_CORES,
                hk: int = 16, xbufs: int = 3, ebufs: int = 2,
                pebufs: int = 2, sbufs: int = 2, robufs: int = 3,
                bf16io: bool = False, gp_off: bool = False,
                out_one: bool = False):
    """No-PE layout: partition p = s*64 + wp holds both w rows (2*wp,
    2*wp+1) of h-halfslab s, so the 2x2-window reduction is pure
    free-dim adds on DVE.  Slab = 2*hk h-rows of one image ([128,
    2*hk*C] tile); inputs are just x (no wmat), engines: SP+ACT DMA
    queues, ACT exp, DVE everything else (optionally GPSIMD offload).
    """
    import concourse.bacc as bacc
    import concourse.tile as tile
    from concourse import mybir

    f32 = mybir.dt.float32
    bf16 = mybir.dt.bfloat16
    xdt = bf16 if bf16io else f32
    odt = bf16 if bf16io else f32
    bs = B // ncores
    free = 2 * hk * C            # per-partition slab elems
    nsl = H // (2 * hk)          # slabs per image
    ho_sl = hk // 2              # ho rows per s-group per slab

    nc = bacc.Bacc("TRN2", target_bir_lowering=False, debug=False,
                   num_devices=ncores)
    x_ap = nc.dram_tensor("x", [bs, W, H, C], xdt, kind="ExternalInput").ap()
    out_ap = nc.dram_tensor("out", [bs, WO, HO, C], odt,
                            kind="ExternalOutput").ap()

    with tile.TileContext(nc) as tc:
        with ExitStack() as ctx:
            xpool = ctx.enter_context(tc.tile_pool(name="x", bufs=xbufs))
            epool = ctx.enter_context(tc.tile_pool(name="e", bufs=ebufs))
            pepool = ctx.enter_context(tc.tile_pool(name="pe", bufs=pebufs))
            spool = ctx.enter_context(tc.tile_pool(name="s", bufs=sbufs))
            rpool = ctx.enter_context(tc.tile_pool(name="r", bufs=robufs))
            opool = ctx.enter_context(tc.tile_pool(name="o", bufs=robufs))

            # x[b] viewed as (wp, j, h, c): w = 2*wp + j
            for _rep in range(reps):
                for b in range(bs):
                    xv = x_ap[b].rearrange("(wp j) h c -> wp j h c", j=2)
                    for sl in range(nsl):
                        h0 = sl * 2 * hk
                        t3 = xpool.tile([128, free], xdt, tag="t",
                                        name="t3").rearrange(
                            "p (j h c) -> p j h c", j=2, h=hk)
                        for s in range(2):
                            eng = nc.sync if (s == 0) else nc.scalar
                            eng.dma_start(
                                t3[s * 64:(s + 1) * 64],
                                xv[:, :, h0 + s * hk:h0 + (s + 1) * hk, :])
                        e3 = epool.tile([128, free], f32, tag="e",
                                        name="e3").rearrange(
                            "p (j h c) -> p j h c", j=2, h=hk)
                        nc.scalar.activation(
                            e3, t3, mybir.ActivationFunctionType.Exp,
                            scale=float(temp))
                        pe3 = pepool.tile([128, free], f32, tag="pe",
                                          name="pe3").rearrange(
                            "p (j h c) -> p j h c", j=2, h=hk)
                        nc.vector.tensor_mul(pe3, t3, e3)
                        sE = spool.tile([128, free // 2], f32, tag="sE",
                                        name="sE").rearrange(
                            "p (h c) -> p h c", h=hk)
                        sP = spool.tile([128, free // 2], f32, tag="sP",
                                        name="sP").rearrange(
                            "p (h c) -> p h c", h=hk)
                        nc.vector.tensor_add(sE, e3[:, 0], e3[:, 1])
                        if gp_off:
                            nc.gpsimd.tensor_add(sP, pe3[:, 0], pe3[:, 1])
                        else:
                            nc.vector.tensor_add(sP, pe3[:, 0], pe3[:, 1])
                        den = rpool.tile([128, free // 4], f32, tag="den",
                                         name="den").rearrange(
                            "p (h c) -> p h c", h=ho_sl)
                        num = rpool.tile([128, free // 4], f32, tag="num",
                                         name="num").rearrange(
                            "p (h c) -> p h c", h=ho_sl)
                        nc.vector.tensor_add(den, sE[:, 0::2], sE[:, 1::2])
                        if gp_off:
                            nc.gpsimd.tensor_add(
                                num, sP[:, 0::2], sP[:, 1::2])
                        else:
                            nc.vector.tensor_add(
                                num, sP[:, 0::2], sP[:, 1::2])
                        r = rpool.tile([128, free // 4], f32, tag="r",
                                       name="r")
                        nc.vector.reciprocal_approx_fast(r[:], den[:, :, :])
                        o = opool.tile([128, free // 4], odt, tag="o",
                                       name="o")
                        nc.vector.tensor_mul(
                            o.rearrange("p (h c) -> p h c", h=ho_sl), num,
                            r.rearrange("p (h c) -> p h c", h=ho_sl))
                        ho0 = h0 // 2
                        ov = o.rearrange("p (h c) -> p h c", h=ho_sl)
                        if out_one:
                            # single DMA: dst (s wo) = partition index
                            dv = out_ap[b, :, ho0:ho0 + 2 * ho_sl, :]\
                                .rearrange("wo (s hh) c -> (s wo) hh c", s=2)
                            nc.sync.dma_start(dv, ov)
                        else:
                            for s in range(2):
                                nc.sync.dma_start(
                                    out_ap[b, :,
                                           ho0 + s * ho_sl:
                                           ho0 + (s + 1) * ho_sl, :],
                                    ov[s * 64:(s + 1) * 64])
    nc.compile()
    return nc


def _wmat() -> np.ndarray:
    w = np.zeros((2, W, 128), dtype=np.float32)
    for j in range(2):
        w[j, np.arange(W), j * 64 + np.arange(W) // 2] = 1.0
    return w


def _in_maps(x: np.ndarray, ncores: int = N_CORES,
             f16io: bool = False, bf16io: bool = False,
             wmat: bool = True, wdt=None) -> list:
    if f16io:
        x = x.astype(np.float16)
    if bf16io:
        import ml_dtypes
        x = np.ascontiguousarray(x.astype(ml_dtypes.bfloat16))
    shards = np.split(x, ncores, axis=0)
    if not wmat:
        return [{"x": s} for s in shards]
    w = _wmat()
    if wdt is not None:
        w = w.astype(wdt)
    return [{"x": s, "wmat": w} for s in shards]


# Shipped configuration: f16 end-to-end — f16 input transport, f16
# e/pe tensors so the DVE x*e multiply runs in 2x packed mode (the DVE
# is the bottleneck engine), f16 PE matmul contraction (full rate +
# FWL), f16 output transport (halves out-DMA bytes).  fp32 only in
# PSUM accumulation and the recip/final-mul drain.
SHIP_CONFIG = dict(variant="f16", gp_rows_ovr=0, f16io=True,
                   hchunk=32, psbufs=1, xbufs=3, ebufs=2, pebufs=2,
                   sbufs=1, dslab=True)


def _build_ship(temp: float, reps: int = 1, **overrides):
    cfg = dict(SHIP_CONFIG)
    cfg.update(overrides)
    return _build(temp, reps=reps, **cfg)


def _ship_in_maps(x: np.ndarray) -> list:
    xb = np.ascontiguousarray(x.astype(np.float16))
    wmat = _wmat().astype(np.float16)
    return [{"x": s, "wmat": wmat} for s in np.split(xb, N_CORES, axis=0)]


def kernel(x: np.ndarray, temperature: np.ndarray) -> np.ndarray:
    from concourse.bass_utils import run_bass_kernel_spmd

    x = np.ascontiguousarray(np.asarray(x, dtype=np.float32))
    temp = float(np.asarray(temperature, dtype=np.float32).reshape(-1)[0])
    # f16 e=exp(t*x) would overflow (65504) if |t*x| > ~11; spec always
    # has t=1 and |x|<~6, but guard with the bf16-transport f32r build.
    if abs(temp) * float(np.abs(x).max()) > 10.0:
        import ml_dtypes
        nc = _build(temp, reps=1, variant="f32r", gp_rows_ovr=0,
                    bf16in=True, hchunk=32, psbufs=1, xbufs=4, ebufs=2,
                    pebufs=2, sbufs=1)
        xb = np.ascontiguousarray(x.astype(ml_dtypes.bfloat16))
        wm = _wmat()
        in_maps = [{"x": s, "wmat": wm}
                   for s in np.split(xb, N_CORES, axis=0)]
    else:
        nc = _build_ship(temp, reps=1)
        in_maps = _ship_in_maps(x)
    last_exc = None
    for _attempt in range(2):
        try:
            res = run_bass_kernel_spmd(nc, in_maps,
                                       core_ids=list(range(N_CORES)))
            break
        except Exception as exc:  # one retry in case of a wedged device
            last_exc = exc
    else:
        raise last_exc
    out = np.concatenate([res.results[i]["out"] for i in range(N_CORES)],
                         axis=0)
    return out.astype(np.float32)



# revision 13
# speedup vs baseline: 1.0442x; 1.0442x over previous
"""AttMaxPool2D (2x2 softmax-attention pooling) Trainium2 Bass kernel.

out[b, wo, ho, c] = sum_i p_i * exp(t*p_i) / sum_i exp(t*p_i)
over the 4 elements p_i of each 2x2 window of x[b, :, :, c] (softmax-
weighted pooling; jax.nn.softmax's max-subtraction cancels analytically).

Sharding: pure batch data-parallel, 8 cores x 4 images, no collectives.

Shipped configuration (SHIP_CONFIG: variant="f16", f16io=True): f16
end-to-end --
 - x ships/stores in HBM as f16 (16.8MB/core), output as f16 (4.2MB);
   fp32 only in PSUM accumulation and the recip/final-mul drain.
 - SBUF tiles hold [w:128(partitions), (h:64, c:128)(free)] double
   slabs (dslab=True): one ACT exp + one DVE mul instruction per 64
   h-rows (fewer per-op inits on the bottleneck DVE; ~0.7% in A/B).
   Splitting that mul back into per-j2 halves (dsmul2) measured 3.7%
   WORSE — DVE op-count dominates sub-tile semaphore granularity.
 - e = exp(t*x) on ScalarE (f16 -> f16, 1x rate: ~57us/core busy).
 - pe = x*e on VectorE as f16 tensor_tensor -> runs in the DVE 2x_1P
   packed mode (the key win over the old bf16-in/f32r config whose
   mixed-dtype mul ran 1x).
 - 2x2-window contraction on the PE as f16 matmuls against a 0/1
   pair-sum weight matrix (full rate + FWL), PSUM-accumulating h-pairs.
 - r = 1/den (custom-DVE fast recip, fp32 from PSUM), out = num*r on
   VectorE (f16 out).
 - No GPSIMD anywhere (shares/locks the DVE SBUF port; measured harmful
   in every layout).

Engine model per core per rep (DVE @0.96GHz is the bottleneck and is
provably at its floor for this decomposition):
  DVE: pe-mul 33.7k cyc (2 elem/cyc packed; port floor) + recip 17.3k
       + final-mul 17.3k (both 1 elem/cyc through the single 32-bit
       DVE<->PSUM read port; floor) = 68.4k cyc = 71.2us
  ACT: exp 16x(4096+352)/1.2GHz = 59.3us   PE: ~55us   DMA: 21MB ~59us
Measured (paired pipelined-slope reps-delta, see test.py):
  shipped f16 e2e:           71.9us/rep, rel err 8.9e-4
  prior   f32r+gp0+bf16in:   ~96-104us/rep, rel err 3.7e-3
  f32r+gp0, fp32 io:         ~125-127us/rep (= fp32-byte DMA floor)
  no-PE same-partition pair layout: ~210us/rep (all-DVE; rejected)
Rejected beyond-floor ideas (all dead-ends, see session notes): ACT
recip offload (exp/recip table sets can't coexist, 2.7us switch), DMA
PSUM drain (no route), GPSIMD anything (port lock), 2x_2P custom recip
(needs 16 slices), finite-difference num (2x ACT or f32-cancellation),
pairwise sigmoid/swish trees (not the same function / more ACT work).
A single reps=1 execution additionally pays ~600us dispatch+launch
overhead through the axon/PJRT stack; that term is environment-, not
kernel-, determined.
"""

import numpy as np
from contextlib import ExitStack

N_CORES = 8
B, W, H, C = 32, 128, 128, 128
BS = B // N_CORES            # images per core
HCHUNK = 16                  # h rows per slab
NPP = H // (2 * HCHUNK)      # psum iterations per image (h-chunk pairs)
WO, HO = W // 2, H // 2
FREE = HCHUNK * C            # slab free size (2048 f32)
PFREE = (HCHUNK // 2) * C    # psum free size (1024 f32)

# rows of each slab's pe-multiply done on GpSimd (of HCHUNK)
GP_PE_ROWS_F32 = 0
GP_PE_ROWS_F32R = 8


def _build(temp: float, reps: int = 1, variant: str = "fp32",
           dma_only: bool = False, no_pe: bool = False,
           gp_rows_ovr: int = None, direct_den: bool = False,
           hchunk: int = HCHUNK, sp_gp: bool = False,
           xbufs: int = 5, ebufs: int = 4, pebufs: int = 4, sbufs: int = 4,
           psbufs: int = None, out_alt: bool = False, robufs: int = 3,
           fused: bool = False, narrow_w: bool = False,
           ncores: int = N_CORES, f16io: bool = False,
           bf16io: bool = False, bf16in: bool = False,
           qbal: bool = False, split_ps: bool = False,
           in3q: bool = False, dslab: bool = False,
           dsmul2: bool = False):
    import concourse.bacc as bacc
    import concourse.tile as tile
    from concourse import mybir

    f32 = mybir.dt.float32
    f16 = mybir.dt.float16
    f32r = mybir.dt.float32r
    use_f32r = variant in ("f32r", "f16")
    edt = {"f32r": f32r, "f16": f16}.get(variant, f32)
    bf16 = mybir.dt.bfloat16
    xdt = bf16 if (bf16io or bf16in) else (f16 if f16io else f32)
    odt = bf16 if bf16io else (f16 if f16io else f32)
    bs = B // ncores

    free = hchunk * C
    pfree = (hchunk // 2) * C
    npp = H // (2 * hchunk)
    nq = pfree // 512
    if psbufs is None:
        psbufs = max(1, 8 // (2 * (pfree // 512 * 1)))
        psbufs = min(psbufs, 2)

    nc = bacc.Bacc("TRN2", target_bir_lowering=False, debug=False,
                   num_devices=ncores)
    x_ap = nc.dram_tensor("x", [bs, W, H, C], xdt, kind="ExternalInput").ap()
    w_ap = nc.dram_tensor("wmat", [2, W, 128], edt,
                          kind="ExternalInput").ap()
    out_ap = nc.dram_tensor("out", [bs, WO, HO, C], odt,
                            kind="ExternalOutput").ap()

    with tile.TileContext(nc) as tc:
        with ExitStack() as ctx:
            wpool = ctx.enter_context(tc.tile_pool(name="w", bufs=1))
            xpool = ctx.enter_context(tc.tile_pool(name="x", bufs=xbufs))
            epool = ctx.enter_context(tc.tile_pool(name="e", bufs=ebufs))
            pepool = ctx.enter_context(tc.tile_pool(name="pe", bufs=pebufs))
            spool = ctx.enter_context(tc.tile_pool(name="s", bufs=sbufs))
            rpool = ctx.enter_context(tc.tile_pool(name="r", bufs=robufs))
            opool = ctx.enter_context(tc.tile_pool(name="o", bufs=robufs))
            pspool = ctx.enter_context(
                tc.tile_pool(name="ps", bufs=psbufs, space="PSUM"))

            wm = wpool.tile([W, 256], edt)
            nc.sync.dma_start(wm[:, 0:128], w_ap[0])
            nc.sync.dma_start(wm[:, 128:256], w_ap[1])

            gp_rows = GP_PE_ROWS_F32R if use_f32r else GP_PE_ROWS_F32
            if gp_rows_ovr is not None:
                gp_rows = gp_rows_ovr
            if fused:
                # hchunk=32 slabs, single big ops, [128,1024] psum granularity
                assert hchunk == 32 and variant == "fp32"
                for _rep in range(reps):
                    for b in range(bs):
                        for sl in range(4):
                            t3 = xpool.tile([128, free], xdt, tag="t",
                                            name="t3").rearrange(
                                "p (h c) -> p h c", h=hchunk)
                            eng = nc.sync if (sl % 2 == 0) else nc.scalar
                            eng.dma_start(
                                t3,
                                x_ap[b, :, sl * hchunk:(sl + 1) * hchunk, :])
                            e3 = epool.tile([128, free], f32, tag="e",
                                            name="e3").rearrange(
                                "p (h c) -> p h c", h=hchunk)
                            nc.scalar.activation(
                                e3, t3, mybir.ActivationFunctionType.Exp,
                                scale=float(temp))
                            pe3 = pepool.tile([128, free], f32, tag="pe",
                                              name="pe3").rearrange(
                                "p (h c) -> p h c", h=hchunk)
                            nc.vector.tensor_mul(pe3, t3, e3)
                            sE = spool.tile([128, free // 2], f32, tag="sE",
                                            name="sE").rearrange(
                                "p (h c) -> p h c", h=hchunk // 2)
                            sP = spool.tile([128, free // 2], f32, tag="sP",
                                            name="sP").rearrange(
                                "p (h c) -> p h c", h=hchunk // 2)
                            nc.vector.tensor_add(
                                sE, e3[:, 0::2, :], e3[:, 1::2, :])
                            if sp_gp:
                                nc.gpsimd.tensor_add(
                                    sP, pe3[:, 0::2, :], pe3[:, 1::2, :])
                            else:
                                nc.vector.tensor_add(
                                    sP, pe3[:, 0::2, :], pe3[:, 1::2, :])
                            den_ps = pspool.tile([128, 1024], f32)
                            num_ps = pspool.tile([128, 1024], f32)
                            for g in range(2):
                                wm_g = wm[:, g * 128:(g + 1) * 128]
                                for q in range(2):
                                    h0 = g * 8 + q * 4
                                    if narrow_w:
                                        # [128,64] weight + dst partition
                                        # offset: halves each LDW (fp32 has
                                        # no fast-weight-load)
                                        ps_sl = (slice(g * 64, (g + 1) * 64),
                                                 slice(q * 512, (q + 1) * 512))
                                        nc.tensor.matmul(
                                            den_ps[ps_sl], wm[:, 0:64],
                                            sE[:, h0:h0 + 4, :],
                                            start=True, stop=True)
                                        nc.tensor.matmul(
                                            num_ps[ps_sl], wm[:, 0:64],
                                            sP[:, h0:h0 + 4, :],
                                            start=True, stop=True)
                                        continue
                                    ps_sl = (slice(0, 128),
                                             slice(q * 512, (q + 1) * 512))
                                    nc.tensor.matmul(
                                        den_ps[ps_sl], wm_g,
                                        sE[:, h0:h0 + 4, :],
                                        start=(g == 0), stop=(g == 1))
                                    nc.tensor.matmul(
                                        num_ps[ps_sl], wm_g,
                                        sP[:, h0:h0 + 4, :],
                                        start=(g == 0), stop=(g == 1))
                            r = rpool.tile([128, 1024], f32)
                            nc.vector.reciprocal_approx_fast(r[:], den_ps[:])
                            o = opool.tile([128, 1024], odt)
                            nc.vector.tensor_mul(o[:], num_ps[:], r[:])
                            o3 = o.rearrange("p (h c) -> p h c", h=8)
                            for g in range(2):
                                ho0 = sl * 16 + g * 8
                                nc.sync.dma_start(
                                    out_ap[b, :, ho0:ho0 + 8, :],
                                    o3[g * 64:(g + 1) * 64, :, :])
            for _rep in range(reps if not fused else 0):
                for b in range(bs):
                    for pp in range(npp):
                        if split_ps:
                            den_h = [pspool.tile([128, 1024], f32,
                                                 name=f"dh{h}")
                                     for h in range(2)]
                            num_h = [pspool.tile([128, 1024], f32,
                                                 name=f"nh{h}")
                                     for h in range(2)]
                            den_ps = num_ps = None
                        else:
                            den_ps = pspool.tile([128, pfree], f32)
                            num_ps = pspool.tile([128, pfree], f32)
                        if dslab:
                            # one [128, 2*free] slab per pp: single ACT
                            # exp + single DVE mul over both j2 halves
                            # (fewer per-op inits on the bottleneck DVE)
                            assert use_f32r and not dma_only
                            assert gp_rows == 0 and not split_ps
                            t3d = xpool.tile([128, 2 * free], xdt, tag="t",
                                             name="t3").rearrange(
                                "p (h c) -> p h c", h=2 * hchunk)
                            for j2 in range(2):
                                hp = 2 * pp + j2
                                eng = (nc.sync if (hp % 2 == 0)
                                       else nc.scalar)
                                eng.dma_start(
                                    t3d[:, j2 * hchunk:(j2 + 1) * hchunk,
                                        :],
                                    x_ap[b, :,
                                         hp * hchunk:(hp + 1) * hchunk, :])
                            e3d = epool.tile([128, 2 * free], edt, tag="e",
                                             name="e3").rearrange(
                                "p (h c) -> p h c", h=2 * hchunk)
                            nc.scalar.activation(
                                e3d, t3d, mybir.ActivationFunctionType.Exp,
                                scale=float(temp))
                            pe3d = pepool.tile([128, 2 * free], edt,
                                               tag="pe",
                                               name="pe3").rearrange(
                                "p (h c) -> p h c", h=2 * hchunk)
                            if dsmul2:
                                # split the mul per j2 half so the first
                                # half's num-matmuls (and thus the fmul)
                                # unblock ~2us earlier; costs +58 cyc/pp
                                for j2m in range(2):
                                    sl = slice(j2m * hchunk,
                                               (j2m + 1) * hchunk)
                                    nc.vector.tensor_mul(
                                        pe3d[:, sl, :], t3d[:, sl, :],
                                        e3d[:, sl, :])
                            else:
                                nc.vector.tensor_mul(pe3d, t3d, e3d)
                        for j2 in range(2):
                            hp = 2 * pp + j2
                            hoff = 0
                            if dslab:
                                e3, pe3 = e3d, pe3d
                                hoff = j2 * hchunk
                            else:
                                t3 = xpool.tile([128, free], xdt, tag="t",
                                                name="t3").rearrange(
                                    "p (h c) -> p h c", h=hchunk)
                                if in3q:
                                    eng = [nc.sync, nc.scalar,
                                           nc.gpsimd][(b * npp * 2 + hp)
                                                      % 3]
                                else:
                                    eng = (nc.sync if (hp % 2 == 0)
                                           else nc.scalar)
                                eng.dma_start(
                                    t3,
                                    x_ap[b, :,
                                         hp * hchunk:(hp + 1) * hchunk, :])
                                if dma_only:
                                    continue
                                e3 = epool.tile([128, free], edt, tag="e",
                                                name="e3").rearrange(
                                    "p (h c) -> p h c", h=hchunk)
                                nc.scalar.activation(
                                    e3, t3,
                                    mybir.ActivationFunctionType.Exp,
                                    scale=float(temp))
                                pe3 = pepool.tile([128, free], edt,
                                                  tag="pe",
                                                  name="pe3").rearrange(
                                    "p (h c) -> p h c", h=hchunk)
                                k = hchunk - gp_rows
                                nc.vector.tensor_mul(
                                    pe3[:, :k, :], t3[:, :k, :],
                                    e3[:, :k, :])
                                if gp_rows:
                                    nc.gpsimd.tensor_mul(
                                        pe3[:, k:, :], t3[:, k:, :],
                                        e3[:, k:, :])
                            wm_j = wm[:, j2 * 128:(j2 + 1) * 128]
                            if use_f32r:
                                for q in range(nq):
                                    for dh in range(2):
                                        h0 = hoff + q * 8 + dh
                                        h1 = hoff + q * 8 + 8
                                        if split_ps:
                                            dtile = den_h[q // 2]
                                            ntile = num_h[q // 2]
                                            ps_sl = (slice(0, 128),
                                                     slice((q % 2) * 512,
                                                           (q % 2 + 1) * 512))
                                        else:
                                            dtile = den_ps
                                            ntile = num_ps
                                            ps_sl = (slice(0, 128),
                                                     slice(q * 512,
                                                           (q + 1) * 512))
                                        st = (j2 == 0 and dh == 0)
                                        sp = (j2 == 1 and dh == 1)
                                        nc.tensor.matmul(
                                            dtile[ps_sl], wm_j,
                                            e3[:, h0:h1:2, :],
                                            start=st, stop=sp)
                                        nc.tensor.matmul(
                                            ntile[ps_sl], wm_j,
                                            pe3[:, h0:h1:2, :],
                                            start=st, stop=sp)
                            else:
                                sE = spool.tile([128, pfree], f32, tag="sE",
                                                name="sE").rearrange(
                                    "p (h c) -> p h c", h=hchunk // 2)
                                sP = spool.tile([128, pfree], f32, tag="sP",
                                                name="sP").rearrange(
                                    "p (h c) -> p h c", h=hchunk // 2)
                                if not direct_den:
                                    nc.vector.tensor_add(
                                        sE, e3[:, 0::2, :], e3[:, 1::2, :])
                                if sp_gp:
                                    nc.gpsimd.tensor_add(
                                        sP, pe3[:, 0::2, :], pe3[:, 1::2, :])
                                else:
                                    nc.vector.tensor_add(
                                        sP, pe3[:, 0::2, :], pe3[:, 1::2, :])
                                if no_pe:
                                    ho0 = hp * (hchunk // 2)
                                    nc.sync.dma_start(
                                        out_ap[b, :,
                                               ho0:ho0 + hchunk // 2, :],
                                        sE[0:64, :, :])
                                    continue
                                for q in range(nq):
                                    ps_sl = (slice(0, 128),
                                             slice(q * 512, (q + 1) * 512))
                                    q0, q1 = q * 4, (q + 1) * 4
                                    if direct_den:
                                        for dh in range(2):
                                            h0 = q * 8 + dh
                                            h1 = q * 8 + 8
                                            nc.tensor.matmul(
                                                den_ps[ps_sl], wm_j,
                                                e3[:, h0:h1:2, :],
                                                start=(j2 == 0 and dh == 0),
                                                stop=(j2 == 1 and dh == 1))
                                    else:
                                        nc.tensor.matmul(
                                            den_ps[ps_sl], wm_j,
                                            sE[:, q0:q1, :],
                                            start=(j2 == 0), stop=(j2 == 1))
                                    nc.tensor.matmul(
                                        num_ps[ps_sl], wm_j, sP[:, q0:q1, :],
                                        start=(j2 == 0), stop=(j2 == 1))
                        if no_pe:
                            continue
                        if dma_only:
                            for j2 in range(2):
                                ho0 = pp * hchunk + j2 * (hchunk // 2)
                                nc.sync.dma_start(
                                    out_ap[b, :, ho0:ho0 + hchunk // 2, :],
                                    t3[j2 * 64:(j2 + 1) * 64,
                                       0:hchunk // 2, :])
                            continue
                        if split_ps:
                            # per-half drain: recip/final of half A overlap
                            # half B's matmuls and release PSUM early
                            for hh in range(2):
                                r = rpool.tile([128, 1024], f32,
                                               name=f"r{hh}")
                                nc.vector.reciprocal_approx_fast(
                                    r[:], den_h[hh][:])
                                o = opool.tile([128, 1024], odt,
                                               name=f"o{hh}")
                                nc.vector.tensor_mul(o[:], num_h[hh][:],
                                                     r[:])
                                o3h = o.rearrange("p (h c) -> p h c", h=8)
                                for j2 in range(2):
                                    ho0 = (2 * pp + j2) * (hchunk // 2) \
                                        + hh * 8
                                    oeng = nc.gpsimd if qbal else nc.sync
                                    oeng.dma_start(
                                        out_ap[b, :, ho0:ho0 + 8, :],
                                        o3h[j2 * 64:(j2 + 1) * 64, :, :])
                            continue
                        r = rpool.tile([128, pfree], f32)
                        nc.vector.reciprocal_approx_fast(r[:], den_ps[:])
                        o = opool.tile([128, pfree], odt)
                        nc.vector.tensor_mul(o[:], num_ps[:], r[:])
                        o3 = o.rearrange("p (h c) -> p h c", h=hchunk // 2)
                        for j2 in range(2):
                            ho0 = pp * hchunk + j2 * (hchunk // 2)
                            if qbal:
                                oeng = nc.gpsimd
                            else:
                                oeng = (nc.scalar
                                        if (out_alt and j2 == 1) else nc.sync)
                            oeng.dma_start(
                                out_ap[b, :, ho0:ho0 + hchunk // 2, :],
                                o3[j2 * 64:(j2 + 1) * 64, :, :])
    nc.compile()
    return nc


def _build_diag(temp: float, reps: int = 1, mode: str = "actonly",
                ncores: int = N_CORES, xbufs: int = 4, ebufs: int = 3):
    """Diagnostic bodies matching the shipped kernel's DMA pattern.
    mode=actonly: in-DMA (bf16) + ACT exp only -> measures ACT
    throughput overlapped with input DMA.  mode=dmabf: in-DMA (bf16) +
    out-DMA (f32, from a memset tile) -> ship's byte traffic, zero
    compute."""
    import concourse.bacc as bacc
    import concourse.tile as tile
    from concourse import mybir

    f32 = mybir.dt.float32
    f32r = mybir.dt.float32r
    bf16 = mybir.dt.bfloat16
    bs = B // ncores
    hchunk = 32
    free = hchunk * C
    pfree = (hchunk // 2) * C
    npp = H // (2 * hchunk)

    nc = bacc.Bacc("TRN2", target_bir_lowering=False, debug=False,
                   num_devices=ncores)
    x_ap = nc.dram_tensor("x", [bs, W, H, C], bf16,
                          kind="ExternalInput").ap()
    out_ap = nc.dram_tensor("out", [bs, WO, HO, C], f32,
                            kind="ExternalOutput").ap()
    with tile.TileContext(nc) as tc:
        with ExitStack() as ctx:
            xpool = ctx.enter_context(tc.tile_pool(name="x", bufs=xbufs))
            epool = ctx.enter_context(tc.tile_pool(name="e", bufs=ebufs))
            opool = ctx.enter_context(tc.tile_pool(name="o", bufs=1))
            od = opool.tile([128, pfree], f32)
            nc.vector.memset(od, 1.0)
            for _rep in range(reps):
                for b in range(bs):
                    for pp in range(npp):
                        for j2 in range(2):
                            hp = 2 * pp + j2
                            t3 = xpool.tile([128, free], bf16, tag="t",
                                            name="t3").rearrange(
                                "p (h c) -> p h c", h=hchunk)
                            eng = nc.sync if (hp % 2 == 0) else nc.scalar
                            eng.dma_start(
                                t3,
                                x_ap[b, :, hp * hchunk:(hp + 1) * hchunk, :])
                            if mode == "actonly":
                                e3 = epool.tile([128, free], f32r, tag="e",
                                                name="e3").rearrange(
                                    "p (h c) -> p h c", h=hchunk)
                                nc.scalar.activation(
                                    e3, t3,
                                    mybir.ActivationFunctionType.Exp,
                                    scale=float(temp))
                        if mode == "dmabf":
                            o3 = od.rearrange("p (h c) -> p h c",
                                              h=hchunk // 2)
                            for j2 in range(2):
                                ho0 = pp * hchunk + j2 * (hchunk // 2)
                                nc.sync.dma_start(
                                    out_ap[b, :, ho0:ho0 + hchunk // 2, :],
                                    o3[j2 * 64:(j2 + 1) * 64, :, :])
                    if mode == "actonly":
                        # minimal out write to keep the NEFF valid
                        nc.sync.dma_start(out_ap[b, 0:64, 0:1, :],
                                          od[0:64, 0:128].rearrange(
                                              "p (h c) -> p h c", h=1))
    nc.compile()
    return nc


def _build_nope(temp: float, reps: int = 1, ncores: int = N_CORES,
                hk: int = 16, xbufs: int = 3, ebufs: int = 2,
                pebufs: int = 2, sbufs: int = 2, robufs: int = 3,
                bf16io: bool = False, gp_off: bool = False,
                out_one: bool = False):
    """No-PE layout: partition p = s*64 + wp holds both w rows (2*wp,
    2*wp+1) of h-halfslab s, so the 2x2-window reduction is pure
    free-dim adds on DVE.  Slab = 2*hk h-rows of one image ([128,
    2*hk*C] tile); inputs are just x (no wmat), engines: SP+ACT DMA
    queues, ACT exp, DVE everything else (optionally GPSIMD offload).
    """
    import concourse.bacc as bacc
    import concourse.tile as tile
    from concourse import mybir

    f32 = mybir.dt.float32
    bf16 = mybir.dt.bfloat16
    xdt = bf16 if bf16io else f32
    odt = bf16 if bf16io else f32
    bs = B // ncores
    free = 2 * hk * C            # per-partition slab elems
    nsl = H // (2 * hk)          # slabs per image
    ho_sl = hk // 2              # ho rows per s-group per slab

    nc = bacc.Bacc("TRN2", target_bir_lowering=False, debug=False,
                   num_devices=ncores)
    x_ap = nc.dram_tensor("x", [bs, W, H, C], xdt, kind="ExternalInput").ap()
    out_ap = nc.dram_tensor("out", [bs, WO, HO, C], odt,
                            kind="ExternalOutput").ap()

    with tile.TileContext(nc) as tc:
        with ExitStack() as ctx:
            xpool = ctx.enter_context(tc.tile_pool(name="x", bufs=xbufs))
            epool = ctx.enter_context(tc.tile_pool(name="e", bufs=ebufs))
            pepool = ctx.enter_context(tc.tile_pool(name="pe", bufs=pebufs))
            spool = ctx.enter_context(tc.tile_pool(name="s", bufs=sbufs))
            rpool = ctx.enter_context(tc.tile_pool(name="r", bufs=robufs))
            opool = ctx.enter_context(tc.tile_pool(name="o", bufs=robufs))

            # x[b] viewed as (wp, j, h, c): w = 2*wp + j
            for _rep in range(reps):
                for b in range(bs):
                    xv = x_ap[b].rearrange("(wp j) h c -> wp j h c", j=2)
                    for sl in range(nsl):
                        h0 = sl * 2 * hk
                        t3 = xpool.tile([128, free], xdt, tag="t",
                                        name="t3").rearrange(
                            "p (j h c) -> p j h c", j=2, h=hk)
                        for s in range(2):
                            eng = nc.sync if (s == 0) else nc.scalar
                            eng.dma_start(
                                t3[s * 64:(s + 1) * 64],
                                xv[:, :, h0 + s * hk:h0 + (s + 1) * hk, :])
                        e3 = epool.tile([128, free], f32, tag="e",
                                        name="e3").rearrange(
                            "p (j h c) -> p j h c", j=2, h=hk)
                        nc.scalar.activation(
                            e3, t3, mybir.ActivationFunctionType.Exp,
                            scale=float(temp))
                        pe3 = pepool.tile([128, free], f32, tag="pe",
                                          name="pe3").rearrange(
                            "p (j h c) -> p j h c", j=2, h=hk)
                        nc.vector.tensor_mul(pe3, t3, e3)
                        sE = spool.tile([128, free // 2], f32, tag="sE",
                                        name="sE").rearrange(
                            "p (h c) -> p h c", h=hk)
                        sP = spool.tile([128, free // 2], f32, tag="sP",
                                        name="sP").rearrange(
                            "p (h c) -> p h c", h=hk)
                        nc.vector.tensor_add(sE, e3[:, 0], e3[:, 1])
                        if gp_off:
                            nc.gpsimd.tensor_add(sP, pe3[:, 0], pe3[:, 1])
                        else:
                            nc.vector.tensor_add(sP, pe3[:, 0], pe3[:, 1])
                        den = rpool.tile([128, free // 4], f32, tag="den",
                                         name="den").rearrange(
                            "p (h c) -> p h c", h=ho_sl)
                        num = rpool.tile([128, free // 4], f32, tag="num",
                                         name="num").rearrange(
                            "p (h c) -> p h c", h=ho_sl)
                        nc.vector.tensor_add(den, sE[:, 0::2], sE[:, 1::2])
                        if gp_off:
                            nc.gpsimd.tensor_add(
                                num, sP[:, 0::2], sP[:, 1::2])
                        else:
                            nc.vector.tensor_add(
                                num, sP[:, 0::2], sP[:, 1::2])
                        r = rpool.tile([128, free // 4], f32, tag="r",
                                       name="r")
                        nc.vector.reciprocal_approx_fast(r[:], den[:, :, :])
                        o = opool.tile([128, free // 4], odt, tag="o",
                                       name="o")
                        nc.vector.tensor_mul(
                            o.rearrange("p (h c) -> p h c", h=ho_sl), num,
                            r.rearrange("p (h c) -> p h c", h=ho_sl))
                        ho0 = h0 // 2
                        ov = o.rearrange("p (h c) -> p h c", h=ho_sl)
                        if out_one:
                            # single DMA: dst (s wo) = partition index
                            dv = out_ap[b, :, ho0:ho0 + 2 * ho_sl, :]\
                                .rearrange("wo (s hh) c -> (s wo) hh c", s=2)
                            nc.sync.dma_start(dv, ov)
                        else:
                            for s in range(2):
                                nc.sync.dma_start(
                                    out_ap[b, :,
                                           ho0 + s * ho_sl:
                                           ho0 + (s + 1) * ho_sl, :],
                                    ov[s * 64:(s + 1) * 64])
    nc.compile()
    return nc


def _wmat() -> np.ndarray:
    w = np.zeros((2, W, 128), dtype=np.float32)
    for j in range(2):
        w[j, np.arange(W), j * 64 + np.arange(W) // 2] = 1.0
    return w


def _in_maps(x: np.ndarray, ncores: int = N_CORES,
             f16io: bool = False, bf16io: bool = False,
             wmat: bool = True, wdt=None) -> list:
    if f16io:
        x = x.astype(np.float16)
    if bf16io:
        import ml_dtypes
        x = np.ascontiguousarray(x.astype(ml_dtypes.bfloat16))
    shards = np.split(x, ncores, axis=0)
    if not wmat:
        return [{"x": s} for s in shards]
    w = _wmat()
    if wdt is not None:
        w = w.astype(wdt)
    return [{"x": s, "wmat": w} for s in shards]


# Shipped configuration: f16 end-to-end — f16 input transport, f16
# e/pe tensors so the DVE x*e multiply runs in 2x packed mode (the DVE
# is the bottleneck engine), f16 PE matmul contraction (full rate +
# FWL), f16 output transport (halves out-DMA bytes).  fp32 only in
# PSUM accumulation and the recip/final-mul drain.
SHIP_CONFIG = dict(variant="f16", gp_rows_ovr=0, f16io=True,
                   hchunk=32, psbufs=1, xbufs=3, ebufs=2, pebufs=2,
                   sbufs=1, dslab=True)


def _build_ship(temp: float, reps: int = 1, **overrides):
    cfg = dict(SHIP_CONFIG)
    cfg.update(overrides)
    return _build(temp, reps=reps, **cfg)


def _ship_in_maps(x: np.ndarray) -> list:
    xb = np.ascontiguousarray(x.astype(np.float16))
    wmat = _wmat().astype(np.float16)
    return [{"x": s, "wmat": wmat} for s in np.split(xb, N_CORES, axis=0)]


def kernel(x: np.ndarray, temperature: np.ndarray) -> np.ndarray:
    from concourse.bass_utils import run_bass_kernel_spmd

    x = np.ascontiguousarray(np.asarray(x, dtype=np.float32))
    temp = float(np.asarray(temperature, dtype=np.float32).reshape(-1)[0])
    # f16 e=exp(t*x) would overflow (65504) if |t*x| > ~11; spec always
    # has t=1 and |x|<~6, but guard with the bf16-transport f32r build.
    if abs(temp) * float(np.abs(x).max()) > 10.0:
        import ml_dtypes
        nc = _build(temp, reps=1, variant="f32r", gp_rows_ovr=0,
                    bf16in=True, hchunk=32, psbufs=1, xbufs=4, ebufs=2,
                    pebufs=2, sbufs=1)
        xb = np.ascontiguousarray(x.astype(ml_dtypes.bfloat16))
        wm = _wmat()
        in_maps = [{"x": s, "wmat": wm}
                   for s in np.split(xb, N_CORES, axis=0)]
    else:
        nc = _build_ship(temp, reps=1)
        in_maps = _ship_in_maps(x)
    last_exc = None
    for _attempt in range(2):
        try:
            res = run_bass_kernel_spmd(nc, in_maps,
                                       core_ids=list(range(N_CORES)))
            break
        except Exception as exc:  # one retry in case of a wedged device
            last_exc = exc
    else:
        raise last_exc
    out = np.concatenate([res.results[i]["out"] for i in range(N_CORES)],
                         axis=0)
    return out.astype(np.float32)

